# revision 4
# baseline (speedup 1.0000x reference)
"""Binary Conv2d (sign-act 3x3 binary conv + RPReLU + residual) on 8 trn2 NeuronCores.

Reference computation (forward values):
  a  = sign(x + move0_bias)                       # {-1,0,+1}
  bw = scale_o * sign(conv_w), scale_o = mean |conv_w| over (I,KH,KW)
  z  = conv2d(a, bw, pad=1) + pr_bias0
  y  = where(z>=0, z, alpha*z) + pr_bias1 + x

Strategy: data-parallel over batch (16 imgs -> 2 per core). Conv as 9 shifted
matmuls (taps) x 2 input-channel chunks accumulated in PSUM; activations are
sign values stored bf16 in a zero-bordered 66x66 padded tile per (img, chunk);
weights are sign(w) bf16 (exact) with per-output-channel scale folded into the
epilogue:
  y = Relu((1-a)*(s*psum + b0)) + (a*s*psum + a*b0 + b1) + x      (valid a<1)
"""

import sys
for _p in ("/opt/trn_rl_repo",):
    if _p not in sys.path:
        sys.path.append(_p)

from contextlib import ExitStack

import numpy as np
import ml_dtypes

import concourse.bass as bass
import concourse.tile as tile
from concourse import bacc, mybir
from concourse import bass_utils

N_CORES = 8
B, C, H, W = 16, 256, 64, 64
K = 3
BPC = B // N_CORES            # imgs per core
NCH = C // 128                # channel chunks (2)
PW = W + 2                    # padded width 66
PH = H + 2                    # padded height 66
PSP = PH * PW                 # padded spatial 4356
SP = H * W                    # spatial 4096
RB = 8                        # out rows per block
NBLK = H // RB                # 8 blocks
NBE = RB * W                  # 512 block elems
NTAP = K * K

F32 = mybir.dt.float32
BF16 = mybir.dt.bfloat16

_CACHE = {}


def _build_program():
    nc = bacc.Bacc(
        "TRN2",
        target_bir_lowering=False,
        debug=False,
        enable_asserts=False,
        num_devices=N_CORES,
    )
    x_d = nc.dram_tensor("x", [BPC, C, H, W], F32, kind="ExternalInput").ap()
    # lhsT pack: [128, ic(2) * tap(9) * oc(2) * 128]
    w_d = nc.dram_tensor("w", [128, NCH * NTAP * NCH * 128], BF16,
                         kind="ExternalInput").ap()
    mb_d = nc.dram_tensor("mb", [C, 1], F32, kind="ExternalInput").ap()
    # epilogue constants: [C, 4] = [sA, bA, sV, bVb]
    epi_d = nc.dram_tensor("epi", [C, 4], F32, kind="ExternalInput").ap()
    y_d = nc.dram_tensor("y", [BPC, C, H, W], F32, kind="ExternalOutput").ap()

    with tile.TileContext(nc) as tc:
        _kernel(tc, y_d, x_d, w_d, mb_d, epi_d)
    nc.compile()
    return nc


def _kernel(tc, y_d, x_d, w_d, mb_d, epi_d):
    nc = tc.nc
    ctx = ExitStack()
    with ctx:
        const = ctx.enter_context(tc.tile_pool(name="const", bufs=1))
        xpool = ctx.enter_context(tc.tile_pool(name="x", bufs=1))
        apool = ctx.enter_context(tc.tile_pool(name="act", bufs=1))
        work = ctx.enter_context(tc.tile_pool(name="work", bufs=3))
        psum = ctx.enter_context(tc.tile_pool(name="psum", bufs=4, space="PSUM"))

        # --- constants ---
        wt = const.tile([128, NCH * NTAP * NCH * 128], BF16, tag="wt")
        nc.sync.dma_start(out=wt[:], in_=w_d[:])

        mbt = [const.tile([128, 1], F32, tag=f"mb{ic}", name=f"mbt{ic}")
               for ic in range(NCH)]
        for ic in range(NCH):
            nc.sync.dma_start(out=mbt[ic][:], in_=mb_d[ic * 128:(ic + 1) * 128])
        ept = [const.tile([128, 4], F32, tag=f"ep{oc}", name=f"ept{oc}")
               for oc in range(NCH)]
        for oc in range(NCH):
            nc.sync.dma_start(out=ept[oc][:], in_=epi_d[oc * 128:(oc + 1) * 128])

        # --- x load + sign activation into padded tiles ---
        x_flat = x_d.rearrange("b c h w -> b c (h w)")
        y_flat = y_d.rearrange("b c h w -> b c (h w)")
        xt = {}   # (b, ch) -> [128, 4096] f32
        at = {}   # (b, ch) -> [128, 4356] bf16 padded sign
        NG = 4    # dma/sign row-groups per chunk
        GR = H // NG
        for b in range(BPC):
            for ic in range(NCH):
                xt[b, ic] = xpool.tile([128, SP], F32, tag=f"xt{b}{ic}", name=f"xt{b}{ic}")
                at[b, ic] = apool.tile([128, PSP], BF16, tag=f"at{b}{ic}", name=f"at{b}{ic}")
        # zero the borders of padded tiles (interior is fully written by Sign)
        for b in range(BPC):
            for ic in range(NCH):
                a3 = at[b, ic][:].rearrange("p (h w) -> p h w", w=PW)
                nc.gpsimd.memset(a3[:, 0:1, :], 0.0)
                nc.gpsimd.memset(a3[:, PH - 1:PH, :], 0.0)
                nc.gpsimd.memset(a3[:, 1:PH - 1, 0:1], 0.0)
                nc.gpsimd.memset(a3[:, 1:PH - 1, PW - 1:PW], 0.0)
        for b in range(BPC):
            for g in range(NG):
                for ic in range(NCH):
                    xs = xt[b, ic][:, g * GR * W:(g + 1) * GR * W]
                    nc.sync.dma_start(
                        out=xs,
                        in_=x_flat[b, ic * 128:(ic + 1) * 128,
                                   g * GR * W:(g + 1) * GR * W])
                    a3 = at[b, ic][:].rearrange("p (h w) -> p h w", w=PW)
                    x3 = xs.rearrange("p (h w) -> p h w", w=W)
                    nc.scalar.activation(
                        a3[:, 1 + g * GR:1 + (g + 1) * GR, 1:1 + W], x3,
                        mybir.ActivationFunctionType.Sign,
                        bias=mbt[ic][:], scale=1.0)

        # --- conv blocks ---
        for b in range(BPC):
            for oc in range(NCH):
                for rb in range(NBLK):
                    pt = psum.tile([128, NBE], F32, tag="pt")
                    first = True
                    for ic in range(NCH):
                        a3 = at[b, ic][:].rearrange("p (h w) -> p h w", w=PW)
                        for kh in range(K):
                            for kw in range(K):
                                t = kh * K + kw
                                wslice = wt[:, ((ic * NTAP + t) * NCH + oc)
                                            * 128:((ic * NTAP + t) * NCH + oc + 1) * 128]
                                rhs = a3[:, rb * RB + kh:rb * RB + kh + RB,
                                         kw:kw + W]
                                last = (ic == NCH - 1 and t == NTAP - 1)
                                nc.tensor.matmul(pt[:], wslice, rhs,
                                                 start=first, stop=last)
                                first = False
                    # epilogue: y = Relu(sA*p + bA) + (sV*p + bVb) + x
                    ep = ept[oc]
                    r = work.tile([128, NBE], F32, tag="r")
                    nc.scalar.activation(r[:], pt[:],
                                         mybir.ActivationFunctionType.Relu,
                                         bias=ep[:, 1:2], scale=ep[:, 0:1])
                    v = work.tile([128, NBE], F32, tag="v")
                    nc.vector.tensor_scalar(
                        out=v[:], in0=pt[:], scalar1=ep[:, 2:3],
                        scalar2=ep[:, 3:4],
                        op0=mybir.AluOpType.mult, op1=mybir.AluOpType.add)
                    yt = work.tile([128, NBE], F32, tag="yt")
                    nc.vector.tensor_add(out=yt[:], in0=r[:], in1=v[:])
                    nc.vector.tensor_add(
                        out=yt[:], in0=yt[:],
                        in1=xt[b, oc][:, rb * NBE:(rb + 1) * NBE])
                    nc.sync.dma_start(
                        out=y_flat[b, oc * 128:(oc + 1) * 128,
                                   rb * NBE:(rb + 1) * NBE],
                        in_=yt[:])


def _pack_inputs(x, move0_bias, conv_w, pr_bias0, prelu_alpha, pr_bias1):
    """Host-side prep: weight binarization + epilogue constant folding."""
    f32 = np.float32
    w = conv_w.astype(f32)
    scale = np.abs(w).mean(axis=(1, 2, 3)).astype(f32)          # (O,)
    ws = np.sign(w).astype(ml_dtypes.bfloat16)                  # (O,I,KH,KW)
    # lhsT[k=p(ic), ic, tap, oc, m] = ws[oc*128+m, ic*128+p, kh, kw]
    wsr = ws.reshape(NCH, 128, NCH, 128, K * K)                 # (oc,m,ic,p,t)
    lhsT = wsr.transpose(3, 2, 4, 0, 1)                         # (p,ic,t,oc,m)
    lhsT = np.ascontiguousarray(lhsT).reshape(128, NCH * NTAP * NCH * 128)

    alpha = prelu_alpha.astype(f32).reshape(C)
    assert np.all(alpha < 1.0), "epilogue folding requires alpha < 1"
    b0 = pr_bias0.astype(f32).reshape(C)
    b1 = pr_bias1.astype(f32).reshape(C)
    sA = (1.0 - alpha) * scale
    bA = (1.0 - alpha) * b0
    sV = alpha * scale
    bVb = alpha * b0 + b1
    epi = np.stack([sA, bA, sV, bVb], axis=1).astype(f32)       # (C,4)
    mb = move0_bias.astype(f32).reshape(C, 1)

    common = {"w": lhsT, "mb": mb, "epi": epi}
    in_maps = []
    for i in range(N_CORES):
        m = dict(common)
        m["x"] = np.ascontiguousarray(x[i * BPC:(i + 1) * BPC].astype(f32))
        in_maps.append(m)
    return in_maps


def kernel(x, move0_bias, conv_w, pr_bias0, prelu_alpha, pr_bias1):
    if "nc" not in _CACHE:
        _CACHE["nc"] = _build_program()
    nc = _CACHE["nc"]
    in_maps = _pack_inputs(np.asarray(x), np.asarray(move0_bias),
                           np.asarray(conv_w), np.asarray(pr_bias0),
                           np.asarray(prelu_alpha), np.asarray(pr_bias1))
    res = bass_utils.run_bass_kernel_spmd(nc, in_maps,
                                          core_ids=list(range(N_CORES)))
    _CACHE["last_results"] = res
    out = np.concatenate([res.results[i]["y"] for i in range(N_CORES)], axis=0)
    return out


# revision 10
# speedup vs baseline: 1.5712x; 1.5712x over previous
"""Binary Conv2d (sign-act 3x3 binary conv + RPReLU + residual) on 8 trn2 NeuronCores.

Reference computation (forward values):
  a  = sign(x + move0_bias)                       # {-1,0,+1}
  bw = scale_o * sign(conv_w), scale_o = mean |conv_w| over (I,KH,KW)
  z  = conv2d(a, bw, pad=1) + pr_bias0
  y  = where(z>=0, z, alpha*z) + pr_bias1 + x

Strategy: data-parallel over batch (16 imgs -> 2 per core). Conv as 9 tap
matmuls with fp8e4 DoubleRow (contracts both 128-channel chunks per matmul,
2 MACs/cell/cycle) accumulating in PSUM; activations are exact sign values
in fp8, stored in a zero-bordered 66-wide padded tile per (img); weights are
sign(w) fp8 (exact) with the per-output-channel scale + biases + PReLU folded
into one ScalarE Lrelu activation (per-partition scale/bias/alpha operands):
  q = Lrelu(s*psum + b0; alpha)    ;    y = q [+ b1] + x
"""

import sys
for _p in ("/opt/trn_rl_repo",):
    if _p not in sys.path:
        sys.path.append(_p)

from contextlib import ExitStack

import numpy as np
import ml_dtypes

import concourse.bass as bass
import concourse.tile as tile
from concourse import bacc, mybir
from concourse import bass_utils

N_CORES = 8
B, C, H, W = 16, 256, 64, 64
K = 3
BPC = B // N_CORES            # imgs per core
NCH = C // 128                # channel chunks (2)
PW = W + 2                    # padded width 66
PHR = 72                      # padded rows allocated (>=66, CST 16-aligned)
CST = PHR * PW                # per-chunk stride in act tile (4752, %16==0)
SP = H * W                    # spatial 4096
RB = 8                        # out rows per block
NBLK = H // RB                # 8 blocks
NBE = RB * W                  # 512 block elems
NTAP = K * K

F32 = mybir.dt.float32
FP8 = mybir.dt.float8e4

import os
USE_DR = os.environ.get("K_USE_DR", "1") == "1"      # fp8 DoubleRow matmuls
USE_LRELU = os.environ.get("K_USE_LRELU", "0") == "1"  # broken on HW: Lrelu alpha ignored

_CACHE = {}
LRELU_ALPHA_IMM = 0.25        # set by _pack_inputs (uniform alpha required)


def _build_program(skip_b1: bool):
    nc = bacc.Bacc(
        "TRN2",
        target_bir_lowering=False,
        debug=False,
        enable_asserts=False,
        num_devices=N_CORES,
    )
    x_d = nc.dram_tensor("x", [BPC, C, H, W], F32, kind="ExternalInput").ap()
    # weight pack: [128, tap(9) * oc(2) * icpair(2) * 128] fp8 sign values
    w_d = nc.dram_tensor("w", [128, NTAP * NCH * NCH * 128], FP8,
                         kind="ExternalInput").ap()
    mb_d = nc.dram_tensor("mb", [C, 1], F32, kind="ExternalInput").ap()
    # epilogue constants: [C, 4] = [s, b0, alpha, b1]
    epi_d = nc.dram_tensor("epi", [C, 4], F32, kind="ExternalInput").ap()
    y_d = nc.dram_tensor("y", [BPC, C, H, W], F32, kind="ExternalOutput").ap()

    with tile.TileContext(nc) as tc:
        _kernel(tc, y_d, x_d, w_d, mb_d, epi_d, skip_b1)
    nc.compile()
    return nc


def _kernel(tc, y_d, x_d, w_d, mb_d, epi_d, skip_b1):
    nc = tc.nc
    ctx = ExitStack()
    with ctx:
        const = ctx.enter_context(tc.tile_pool(name="const", bufs=1))
        xpool = ctx.enter_context(tc.tile_pool(name="x", bufs=1))
        apool = ctx.enter_context(tc.tile_pool(name="act", bufs=1))
        work = ctx.enter_context(tc.tile_pool(name="work", bufs=3))
        psum = ctx.enter_context(tc.tile_pool(name="psum", bufs=4, space="PSUM"))

        # --- constants (issue first; small) ---
        wt = const.tile([128, NTAP * NCH * NCH * 128], FP8, tag="wt")
        nc.sync.dma_start(out=wt[:], in_=w_d[:])
        mbt = [const.tile([128, 1], F32, tag=f"mb{ic}", name=f"mbt{ic}")
               for ic in range(NCH)]
        for ic in range(NCH):
            nc.sync.dma_start(out=mbt[ic][:], in_=mb_d[ic * 128:(ic + 1) * 128])
        ept = [const.tile([128, 4], F32, tag=f"ep{oc}", name=f"ept{oc}")
               for oc in range(NCH)]
        for oc in range(NCH):
            nc.sync.dma_start(out=ept[oc][:], in_=epi_d[oc * 128:(oc + 1) * 128])

        # --- x load + sign activation into padded fp8 tiles ---
        x_flat = x_d.rearrange("b c h w -> b c (h w)")
        y_flat = y_d.rearrange("b c h w -> b c (h w)")
        xt = {}   # (b, ic) -> [128, 4096] f32 (residual source)
        at = {}   # b -> [128, 2*CST] fp8 padded sign, chunk ic at offset ic*CST
        for b in range(BPC):
            at[b] = apool.tile([128, NCH * CST], FP8, tag=f"at{b}",
                               name=f"at{b}")
            for ic in range(NCH):
                xt[b, ic] = xpool.tile([128, SP], F32, tag=f"xt{b}{ic}",
                                       name=f"xt{b}{ic}")
        # zero borders (rows 0 & 65, cols 0 & 65 of the 66x66 window)
        for b in range(BPC):
            a4 = at[b][:].rearrange("p (i h w) -> p i h w", i=NCH, w=PW)
            nc.gpsimd.memset(a4[:, :, 0:1, :], 0.0)
            nc.gpsimd.memset(a4[:, :, H + 1:H + 2, :], 0.0)
            nc.gpsimd.memset(a4[:, :, 1:H + 1, 0:1], 0.0)
            nc.gpsimd.memset(a4[:, :, 1:H + 1, PW - 1:PW], 0.0)
        NG = 4    # dma/sign row-groups per chunk
        GR = H // NG
        for b in range(BPC):
            for g in range(NG):
                for ic in range(NCH):
                    xs = xt[b, ic][:, g * GR * W:(g + 1) * GR * W]
                    nc.sync.dma_start(
                        out=xs,
                        in_=x_flat[b, ic * 128:(ic + 1) * 128,
                                   g * GR * W:(g + 1) * GR * W])
                    a4 = at[b][:].rearrange("p (i h w) -> p i h w",
                                            i=NCH, w=PW)
                    x3 = xs.rearrange("p (h w) -> p h w", w=W)
                    nc.scalar.activation(
                        a4[:, ic, 1 + g * GR:1 + (g + 1) * GR, 1:1 + W], x3,
                        mybir.ActivationFunctionType.Sign,
                        bias=mbt[ic][:], scale=1.0)

        # --- conv blocks ---
        for b in range(BPC):
            a4 = at[b][:].rearrange("p (i h w) -> p i h w", i=NCH, w=PW)
            a3 = {ic: at[b][:, ic * CST:(ic + 1) * CST].rearrange(
                "p (h w) -> p h w", w=PW) for ic in range(NCH)}
            for oc in range(NCH):
                for rb in range(NBLK):
                    pt = psum.tile([128, NBE], F32, tag="pt")
                    if USE_DR:
                        for kh in range(K):
                            for kw in range(K):
                                t = kh * K + kw
                                wsl = wt[:, (t * NCH + oc) * NCH
                                         * 128:(t * NCH + oc + 1) * NCH * 128]
                                lhsT = wsl.rearrange("p (i m) -> p i m", i=NCH)
                                rhs = a4[:, :, rb * RB + kh:rb * RB + kh + RB,
                                         kw:kw + W]
                                nc.tensor.matmul(
                                    pt[:], lhsT, rhs,
                                    start=(t == 0), stop=(t == NTAP - 1),
                                    perf_mode=mybir.MatmulPerfMode.DoubleRow)
                    else:
                        first = True
                        for ic in range(NCH):
                            for kh in range(K):
                                for kw in range(K):
                                    t = kh * K + kw
                                    wsl = wt[:, ((t * NCH + oc) * NCH + ic)
                                             * 128:((t * NCH + oc) * NCH + ic
                                                    + 1) * 128]
                                    rhs = a3[ic][:, rb * RB + kh:rb * RB + kh
                                                 + RB, kw:kw + W]
                                    last = (ic == NCH - 1 and t == NTAP - 1)
                                    nc.tensor.matmul(pt[:], wsl, rhs,
                                                     start=first, stop=last)
                                    first = False
                    # epilogue
                    ep = ept[oc]
                    yt = work.tile([128, NBE], F32, tag="yt")
                    if USE_LRELU:
                        q = work.tile([128, NBE], F32, tag="q")
                        nc.scalar.activation(
                            q[:], pt[:], mybir.ActivationFunctionType.Lrelu,
                            bias=ep[:, 1:2], scale=ep[:, 0:1],
                            alpha=LRELU_ALPHA_IMM)
                        if not skip_b1:
                            nc.vector.tensor_scalar(
                                out=q[:], in0=q[:], scalar1=ep[:, 3:4],
                                scalar2=None, op0=mybir.AluOpType.add)
                        nc.vector.tensor_add(
                            out=yt[:], in0=q[:],
                            in1=xt[b, oc][:, rb * NBE:(rb + 1) * NBE])
                    else:
                        # r = Relu((1-a)(s p + b0)); v = a s p + (a b0 + b1)
                        r = work.tile([128, NBE], F32, tag="r")
                        nc.scalar.activation(
                            r[:], pt[:], mybir.ActivationFunctionType.Relu,
                            bias=ep[:, 1:2], scale=ep[:, 0:1])
                        v = work.tile([128, NBE], F32, tag="v")
                        nc.vector.tensor_scalar(
                            out=v[:], in0=pt[:], scalar1=ep[:, 2:3],
                            scalar2=ep[:, 3:4], op0=mybir.AluOpType.mult,
                            op1=mybir.AluOpType.add)
                        nc.vector.tensor_add(out=yt[:], in0=r[:], in1=v[:])
                        nc.vector.tensor_add(
                            out=yt[:], in0=yt[:],
                            in1=xt[b, oc][:, rb * NBE:(rb + 1) * NBE])
                    nc.sync.dma_start(
                        out=y_flat[b, oc * 128:(oc + 1) * 128,
                                   rb * NBE:(rb + 1) * NBE],
                        in_=yt[:])


def _pack_inputs(x, move0_bias, conv_w, pr_bias0, prelu_alpha, pr_bias1):
    """Host-side prep: weight binarization + epilogue constant folding."""
    f32 = np.float32
    w = conv_w.astype(f32)
    scale = np.abs(w).mean(axis=(1, 2, 3)).astype(f32)          # (O,)
    ws = np.sign(w).astype(ml_dtypes.float8_e4m3)               # (O,I,KH,KW)
    # lhsT[k=p, tap, oc, ic, m] = ws[oc*128+m, ic*128+p, kh, kw]
    wsr = ws.reshape(NCH, 128, NCH, 128, NTAP)                  # (oc,m,ic,p,t)
    lhsT = wsr.transpose(3, 4, 0, 2, 1)                         # (p,t,oc,ic,m)
    lhsT = np.ascontiguousarray(lhsT).reshape(128, NTAP * NCH * NCH * 128)

    alpha = prelu_alpha.astype(f32).reshape(C)
    b0 = pr_bias0.astype(f32).reshape(C)
    b1 = pr_bias1.astype(f32).reshape(C)
    if USE_LRELU:
        global LRELU_ALPHA_IMM
        assert np.all(alpha == alpha[0]), "Lrelu path needs uniform alpha"
        LRELU_ALPHA_IMM = float(alpha[0])
        epi = np.stack([scale, b0, alpha, b1], axis=1).astype(f32)
    else:
        assert np.all(alpha < 1.0)
        epi = np.stack([(1 - alpha) * scale, (1 - alpha) * b0,
                        alpha * scale, alpha * b0 + b1], axis=1).astype(f32)
    mb = move0_bias.astype(f32).reshape(C, 1)
    skip_b1 = bool(np.all(b1 == 0.0)) if USE_LRELU else False

    common = {"w": lhsT, "mb": mb, "epi": epi}
    in_maps = []
    for i in range(N_CORES):
        m = dict(common)
        m["x"] = np.ascontiguousarray(x[i * BPC:(i + 1) * BPC].astype(f32))
        in_maps.append(m)
    return in_maps, skip_b1


def kernel(x, move0_bias, conv_w, pr_bias0, prelu_alpha, pr_bias1):
    in_maps, skip_b1 = _pack_inputs(
        np.asarray(x), np.asarray(move0_bias), np.asarray(conv_w),
        np.asarray(pr_bias0), np.asarray(prelu_alpha), np.asarray(pr_bias1))
    key = ("nc", skip_b1)
    if key not in _CACHE:
        _CACHE[key] = _build_program(skip_b1)
    nc = _CACHE[key]
    res = bass_utils.run_bass_kernel_spmd(nc, in_maps,
                                          core_ids=list(range(N_CORES)))
    _CACHE["last_results"] = res
    out = np.concatenate([res.results[i]["y"] for i in range(N_CORES)], axis=0)
    return out


# revision 12
# speedup vs baseline: 1.5743x; 1.0020x over previous
"""Binary Conv2d (sign-act 3x3 binary conv + RPReLU + residual) on 8 trn2 NeuronCores.

Reference computation (forward values):
  a  = sign(x + move0_bias)                       # {-1,0,+1}
  bw = scale_o * sign(conv_w), scale_o = mean |conv_w| over (I,KH,KW)
  z  = conv2d(a, bw, pad=1) + pr_bias0
  y  = where(z>=0, z, alpha*z) + pr_bias1 + x

Strategy: data-parallel over batch (16 imgs -> 2 per core). Conv as 9 tap
matmuls with fp8e4 DoubleRow (contracts both 128-channel chunks per matmul,
2 MACs/cell/cycle) accumulating in PSUM; activations are exact sign values
in fp8, stored in a zero-bordered 66-wide padded tile per (img); weights are
sign(w) fp8 (exact) with the per-output-channel scale + biases + PReLU folded
into one ScalarE Lrelu activation (per-partition scale/bias/alpha operands):
  q = Lrelu(s*psum + b0; alpha)    ;    y = q [+ b1] + x
"""

import sys
for _p in ("/opt/trn_rl_repo",):
    if _p not in sys.path:
        sys.path.append(_p)

from contextlib import ExitStack

import numpy as np
import ml_dtypes

import concourse.bass as bass
import concourse.tile as tile
from concourse import bacc, mybir
from concourse import bass_utils

N_CORES = 8
B, C, H, W = 16, 256, 64, 64
K = 3
BPC = B // N_CORES            # imgs per core
NCH = C // 128                # channel chunks (2)
PW = W + 2                    # padded width 66
PHR = 72                      # padded rows allocated (>=66, CST 16-aligned)
CST = PHR * PW                # per-chunk stride in act tile (4752, %16==0)
SP = H * W                    # spatial 4096
RB = 8                        # out rows per block
NBLK = H // RB                # 8 blocks
NBE = RB * W                  # 512 block elems
NTAP = K * K

F32 = mybir.dt.float32
FP8 = mybir.dt.float8e4

import os
USE_DR = os.environ.get("K_USE_DR", "1") == "1"      # fp8 DoubleRow matmuls
USE_LRELU = os.environ.get("K_USE_LRELU", "0") == "1"  # broken on HW: Lrelu alpha ignored

_CACHE = {}
LRELU_ALPHA_IMM = 0.25        # set by _pack_inputs (uniform alpha required)


def _build_program(skip_b1: bool):
    nc = bacc.Bacc(
        "TRN2",
        target_bir_lowering=False,
        debug=False,
        enable_asserts=False,
        num_devices=N_CORES,
    )
    x_d = nc.dram_tensor("x", [BPC, C, H, W], F32, kind="ExternalInput").ap()
    # weight pack: [128, tap(9) * oc(2) * icpair(2) * 128] fp8 sign values
    w_d = nc.dram_tensor("w", [128, NTAP * NCH * NCH * 128], FP8,
                         kind="ExternalInput").ap()
    mb_d = nc.dram_tensor("mb", [C, 1], F32, kind="ExternalInput").ap()
    # epilogue constants: [C, 4] = [s, b0, alpha, b1]
    epi_d = nc.dram_tensor("epi", [C, 4], F32, kind="ExternalInput").ap()
    y_d = nc.dram_tensor("y", [BPC, C, H, W], F32, kind="ExternalOutput").ap()

    with tile.TileContext(nc) as tc:
        _kernel(tc, y_d, x_d, w_d, mb_d, epi_d, skip_b1)
    nc.compile()
    return nc


def _kernel(tc, y_d, x_d, w_d, mb_d, epi_d, skip_b1):
    nc = tc.nc
    ctx = ExitStack()
    with ctx:
        const = ctx.enter_context(tc.tile_pool(name="const", bufs=1))
        xpool = ctx.enter_context(tc.tile_pool(name="x", bufs=1))
        apool = ctx.enter_context(tc.tile_pool(name="act", bufs=1))
        work = ctx.enter_context(tc.tile_pool(name="work", bufs=3))
        psum = ctx.enter_context(tc.tile_pool(name="psum", bufs=4, space="PSUM"))

        # --- tiles ---
        x_flat = x_d.rearrange("b c h w -> b c (h w)")
        y_flat = y_d.rearrange("b c h w -> b c (h w)")
        xt = {}   # (b, ic) -> [128, 4096] f32 (residual source)
        at = {}   # b -> [128, 2*CST] fp8 padded sign, chunk ic at offset ic*CST
        for b in range(BPC):
            at[b] = apool.tile([128, NCH * CST], FP8, tag=f"at{b}",
                               name=f"at{b}")
            for ic in range(NCH):
                xt[b, ic] = xpool.tile([128, SP], F32, tag=f"xt{b}{ic}",
                                       name=f"xt{b}{ic}")
        wt = const.tile([128, NTAP * NCH * NCH * 128], FP8, tag="wt")
        mbt = [const.tile([128, 1], F32, tag=f"mb{ic}", name=f"mbt{ic}")
               for ic in range(NCH)]
        ept = [const.tile([128, 4], F32, tag=f"ep{oc}", name=f"ept{oc}")
               for oc in range(NCH)]

        # --- DMA trigger order = sync-engine FIFO: startup-critical first ---
        NG = 4    # dma/sign row-groups per chunk
        GR = H // NG

        def dma_x(b, g, ic):
            xs = xt[b, ic][:, g * GR * W:(g + 1) * GR * W]
            nc.sync.dma_start(
                out=xs,
                in_=x_flat[b, ic * 128:(ic + 1) * 128,
                           g * GR * W:(g + 1) * GR * W])

        for ic in range(NCH):
            nc.sync.dma_start(out=mbt[ic][:], in_=mb_d[ic * 128:(ic + 1) * 128])
        dma_x(0, 0, 0)
        dma_x(0, 0, 1)
        nc.sync.dma_start(out=wt[:], in_=w_d[:])
        for oc in range(NCH):
            nc.sync.dma_start(out=ept[oc][:], in_=epi_d[oc * 128:(oc + 1) * 128])
        for b in range(BPC):
            for g in range(NG):
                for ic in range(NCH):
                    if (b, g) != (0, 0):
                        dma_x(b, g, ic)

        # zero borders (rows 0 & 65, cols 0 & 65 of the 66x66 window)
        for b in range(BPC):
            a4 = at[b][:].rearrange("p (i h w) -> p i h w", i=NCH, w=PW)
            nc.gpsimd.memset(a4[:, :, 0:1, :], 0.0)
            nc.gpsimd.memset(a4[:, :, H + 1:H + 2, :], 0.0)
            nc.gpsimd.memset(a4[:, :, 1:H + 1, 0:1], 0.0)
            nc.gpsimd.memset(a4[:, :, 1:H + 1, PW - 1:PW], 0.0)

        # sign activations
        for b in range(BPC):
            for g in range(NG):
                for ic in range(NCH):
                    xs = xt[b, ic][:, g * GR * W:(g + 1) * GR * W]
                    a4 = at[b][:].rearrange("p (i h w) -> p i h w",
                                            i=NCH, w=PW)
                    x3 = xs.rearrange("p (h w) -> p h w", w=W)
                    nc.scalar.activation(
                        a4[:, ic, 1 + g * GR:1 + (g + 1) * GR, 1:1 + W], x3,
                        mybir.ActivationFunctionType.Sign,
                        bias=mbt[ic][:], scale=1.0)

        # --- conv blocks: pairs of 8-row blocks share a 2-bank PSUM tile ---
        NPAIR = NBLK // 2
        PBE = 2 * NBE            # 1024 elems per pair
        for b in range(BPC):
            a4 = at[b][:].rearrange("p (i h w) -> p i h w", i=NCH, w=PW)
            for oc in range(NCH):
                for pr in range(NPAIR):
                    pt = psum.tile([128, PBE], F32, tag="pt")
                    for half in range(2):
                        rb = pr * 2 + half
                        out_half = pt[:, half * NBE:(half + 1) * NBE]
                        for kh in range(K):
                            for kw in range(K):
                                t = kh * K + kw
                                wsl = wt[:, (t * NCH + oc) * NCH
                                         * 128:(t * NCH + oc + 1) * NCH * 128]
                                lhsT = wsl.rearrange("p (i m) -> p i m", i=NCH)
                                rhs = a4[:, :, rb * RB + kh:rb * RB + kh + RB,
                                         kw:kw + W]
                                nc.tensor.matmul(
                                    out_half, lhsT, rhs,
                                    start=(t == 0), stop=(t == NTAP - 1),
                                    perf_mode=mybir.MatmulPerfMode.DoubleRow)
                    # epilogue on the pair:
                    # r = Relu((1-a)(s p + b0)); v = a s p + (a b0 + b1)
                    # y = r + v + x
                    ep = ept[oc]
                    r = work.tile([128, PBE], F32, tag="r")
                    nc.scalar.activation(
                        r[:], pt[:], mybir.ActivationFunctionType.Relu,
                        bias=ep[:, 1:2], scale=ep[:, 0:1])
                    v = work.tile([128, PBE], F32, tag="v")
                    nc.vector.tensor_scalar(
                        out=v[:], in0=pt[:], scalar1=ep[:, 2:3],
                        scalar2=ep[:, 3:4], op0=mybir.AluOpType.mult,
                        op1=mybir.AluOpType.add)
                    yt = work.tile([128, PBE], F32, tag="yt")
                    nc.vector.tensor_add(out=yt[:], in0=r[:], in1=v[:])
                    nc.gpsimd.tensor_add(
                        out=yt[:], in0=yt[:],
                        in1=xt[b, oc][:, pr * PBE:(pr + 1) * PBE])
                    nc.sync.dma_start(
                        out=y_flat[b, oc * 128:(oc + 1) * 128,
                                   pr * PBE:(pr + 1) * PBE],
                        in_=yt[:])


def _pack_inputs(x, move0_bias, conv_w, pr_bias0, prelu_alpha, pr_bias1):
    """Host-side prep: weight binarization + epilogue constant folding."""
    f32 = np.float32
    w = conv_w.astype(f32)
    scale = np.abs(w).mean(axis=(1, 2, 3)).astype(f32)          # (O,)
    ws = np.sign(w).astype(ml_dtypes.float8_e4m3)               # (O,I,KH,KW)
    # lhsT[k=p, tap, oc, ic, m] = ws[oc*128+m, ic*128+p, kh, kw]
    wsr = ws.reshape(NCH, 128, NCH, 128, NTAP)                  # (oc,m,ic,p,t)
    lhsT = wsr.transpose(3, 4, 0, 2, 1)                         # (p,t,oc,ic,m)
    lhsT = np.ascontiguousarray(lhsT).reshape(128, NTAP * NCH * NCH * 128)

    alpha = prelu_alpha.astype(f32).reshape(C)
    b0 = pr_bias0.astype(f32).reshape(C)
    b1 = pr_bias1.astype(f32).reshape(C)
    if USE_LRELU:
        global LRELU_ALPHA_IMM
        assert np.all(alpha == alpha[0]), "Lrelu path needs uniform alpha"
        LRELU_ALPHA_IMM = float(alpha[0])
        epi = np.stack([scale, b0, alpha, b1], axis=1).astype(f32)
    else:
        assert np.all(alpha < 1.0)
        epi = np.stack([(1 - alpha) * scale, (1 - alpha) * b0,
                        alpha * scale, alpha * b0 + b1], axis=1).astype(f32)
    mb = move0_bias.astype(f32).reshape(C, 1)
    skip_b1 = bool(np.all(b1 == 0.0)) if USE_LRELU else False

    common = {"w": lhsT, "mb": mb, "epi": epi}
    in_maps = []
    for i in range(N_CORES):
        m = dict(common)
        m["x"] = np.ascontiguousarray(x[i * BPC:(i + 1) * BPC].astype(f32))
        in_maps.append(m)
    return in_maps, skip_b1


def kernel(x, move0_bias, conv_w, pr_bias0, prelu_alpha, pr_bias1):
    in_maps, skip_b1 = _pack_inputs(
        np.asarray(x), np.asarray(move0_bias), np.asarray(conv_w),
        np.asarray(pr_bias0), np.asarray(prelu_alpha), np.asarray(pr_bias1))
    key = ("nc", skip_b1)
    if key not in _CACHE:
        _CACHE[key] = _build_program(skip_b1)
    nc = _CACHE[key]
    res = bass_utils.run_bass_kernel_spmd(nc, in_maps,
                                          core_ids=list(range(N_CORES)))
    _CACHE["last_results"] = res
    out = np.concatenate([res.results[i]["y"] for i in range(N_CORES)], axis=0)
    return out


# revision 14
# speedup vs baseline: 1.6496x; 1.0478x over previous
"""Binary Conv2d (sign-act 3x3 binary conv + RPReLU + residual) on 8 trn2 NeuronCores.

Reference computation (forward values):
  a  = sign(x + move0_bias)                       # {-1,0,+1}
  bw = scale_o * sign(conv_w), scale_o = mean |conv_w| over (I,KH,KW)
  z  = conv2d(a, bw, pad=1) + pr_bias0
  y  = where(z>=0, z, alpha*z) + pr_bias1 + x

Strategy: data-parallel over batch (16 imgs -> 2 per core). Conv as 9 tap
matmuls with fp8e4 DoubleRow (contracts both 128-channel chunks per matmul,
2 MACs/cell/cycle) accumulating in PSUM; activations are exact sign values
in fp8, stored in a zero-bordered 66-wide padded tile per (img); weights are
sign(w) fp8 (exact) with the per-output-channel scale + biases + PReLU folded
into one ScalarE Lrelu activation (per-partition scale/bias/alpha operands):
  q = Lrelu(s*psum + b0; alpha)    ;    y = q [+ b1] + x
"""

import sys
for _p in ("/opt/trn_rl_repo",):
    if _p not in sys.path:
        sys.path.append(_p)

from contextlib import ExitStack

import numpy as np
import ml_dtypes

import concourse.bass as bass
import concourse.tile as tile
from concourse import bacc, mybir
from concourse import bass_utils

N_CORES = 8
B, C, H, W = 16, 256, 64, 64
K = 3
BPC = B // N_CORES            # imgs per core
NCH = C // 128                # channel chunks (2)
PW = W + 2                    # padded width 66
PHR = 72                      # padded rows allocated (>=66, CST 16-aligned)
CST = PHR * PW                # per-chunk stride in act tile (4752, %16==0)
SP = H * W                    # spatial 4096
RB = 8                        # out rows per block
NBLK = H // RB                # 8 blocks
NBE = RB * W                  # 512 block elems
NTAP = K * K

F32 = mybir.dt.float32
FP8 = mybir.dt.float8e4

import os
USE_DR = os.environ.get("K_USE_DR", "1") == "1"      # fp8 DoubleRow matmuls
USE_LRELU = os.environ.get("K_USE_LRELU", "0") == "1"  # broken on HW: Lrelu alpha ignored

_CACHE = {}
LRELU_ALPHA_IMM = 0.25        # set by _pack_inputs (uniform alpha required)


def _build_program(skip_b1: bool):
    nc = bacc.Bacc(
        "TRN2",
        target_bir_lowering=False,
        debug=False,
        enable_asserts=False,
        num_devices=N_CORES,
    )
    x_d = nc.dram_tensor("x", [BPC, C, H, W], F32, kind="ExternalInput").ap()
    # weight pack: [128, tap(9) * oc(2) * icpair(2) * 128] fp8 sign values
    w_d = nc.dram_tensor("w", [128, NTAP * NCH * NCH * 128], FP8,
                         kind="ExternalInput").ap()
    mb_d = nc.dram_tensor("mb", [C, 1], F32, kind="ExternalInput").ap()
    # epilogue constants: [C, 4] = [s, b0, alpha, b1]
    epi_d = nc.dram_tensor("epi", [C, 4], F32, kind="ExternalInput").ap()
    y_d = nc.dram_tensor("y", [BPC, C, H, W], F32, kind="ExternalOutput").ap()

    with tile.TileContext(nc) as tc:
        _kernel(tc, y_d, x_d, w_d, mb_d, epi_d, skip_b1)
    nc.compile()
    return nc


def _kernel(tc, y_d, x_d, w_d, mb_d, epi_d, skip_b1):
    nc = tc.nc
    ctx = ExitStack()
    with ctx:
        const = ctx.enter_context(tc.tile_pool(name="const", bufs=1))
        xpool = ctx.enter_context(tc.tile_pool(name="x", bufs=1))
        apool = ctx.enter_context(tc.tile_pool(name="act", bufs=1))
        work = ctx.enter_context(tc.tile_pool(name="work", bufs=3))
        psum = ctx.enter_context(tc.tile_pool(name="psum", bufs=4, space="PSUM"))

        # --- tiles ---
        x_flat = x_d.rearrange("b c h w -> b c (h w)")
        y_flat = y_d.rearrange("b c h w -> b c (h w)")
        xt = {}   # (b, ic) -> [128, 4096] f32 (residual source)
        at = {}   # b -> [128, 2*CST] fp8 padded sign, chunk ic at offset ic*CST
        for b in range(BPC):
            at[b] = apool.tile([128, NCH * CST], FP8, tag=f"at{b}",
                               name=f"at{b}")
            for ic in range(NCH):
                xt[b, ic] = xpool.tile([128, SP], F32, tag=f"xt{b}{ic}",
                                       name=f"xt{b}{ic}")
        wt = const.tile([128, NTAP * NCH * NCH * 128], FP8, tag="wt")
        mbt = [const.tile([128, 1], F32, tag=f"mb{ic}", name=f"mbt{ic}")
               for ic in range(NCH)]
        ept = [const.tile([128, 4], F32, tag=f"ep{oc}", name=f"ept{oc}")
               for oc in range(NCH)]

        # --- DMA trigger order = sync-engine FIFO: startup-critical first ---
        # row groups per chunk: sized so block rb only needs groups 0..rb
        GROUPS = [(0, 10), (10, 18), (18, 26), (26, 34), (34, 42), (42, 50),
                  (50, 58), (58, 64)]

        def dma_x(b, g, ic):
            r0, r1 = GROUPS[g]
            xs = xt[b, ic][:, r0 * W:r1 * W]
            nc.sync.dma_start(
                out=xs,
                in_=x_flat[b, ic * 128:(ic + 1) * 128, r0 * W:r1 * W])

        for ic in range(NCH):
            nc.sync.dma_start(out=mbt[ic][:], in_=mb_d[ic * 128:(ic + 1) * 128])
        dma_x(0, 0, 0)
        dma_x(0, 0, 1)
        nc.sync.dma_start(out=wt[:], in_=w_d[:])
        for g in range(1, len(GROUPS)):
            for ic in range(NCH):
                dma_x(0, g, ic)
        for oc in range(NCH):
            nc.sync.dma_start(out=ept[oc][:], in_=epi_d[oc * 128:(oc + 1) * 128])
        for g in range(len(GROUPS)):
            for ic in range(NCH):
                dma_x(1, g, ic)

        # zero borders (rows 0 & 65, cols 0 & 65 of the 66x66 window)
        for b in range(BPC):
            a4 = at[b][:].rearrange("p (i h w) -> p i h w", i=NCH, w=PW)
            nc.gpsimd.memset(a4[:, :, 0:1, :], 0.0)
            nc.gpsimd.memset(a4[:, :, H + 1:H + 2, :], 0.0)
            nc.gpsimd.memset(a4[:, :, 1:H + 1, 0:1], 0.0)
            nc.gpsimd.memset(a4[:, :, 1:H + 1, PW - 1:PW], 0.0)

        # sign activations (per row group, both chunks)
        for b in range(BPC):
            for g in range(len(GROUPS)):
                for ic in range(NCH):
                    r0, r1 = GROUPS[g]
                    xs = xt[b, ic][:, r0 * W:r1 * W]
                    a4 = at[b][:].rearrange("p (i h w) -> p i h w",
                                            i=NCH, w=PW)
                    x3 = xs.rearrange("p (h w) -> p h w", w=W)
                    nc.scalar.activation(
                        a4[:, ic, 1 + r0:1 + r1, 1:1 + W], x3,
                        mybir.ActivationFunctionType.Sign,
                        bias=mbt[ic][:], scale=1.0)

        # --- conv blocks: pairs of 8-row blocks share a 2-bank PSUM tile ---
        NPAIR = NBLK // 2
        PBE = 2 * NBE            # 1024 elems per pair
        for b in range(BPC):
            a4 = at[b][:].rearrange("p (i h w) -> p i h w", i=NCH, w=PW)
            for oc in range(NCH):
                for pr in range(NPAIR):
                    pt = psum.tile([128, PBE], F32, tag="pt")
                    for half in range(2):
                        rb = pr * 2 + half
                        out_half = pt[:, half * NBE:(half + 1) * NBE]
                        for kh in range(K):
                            for kw in range(K):
                                t = kh * K + kw
                                wsl = wt[:, (t * NCH + oc) * NCH
                                         * 128:(t * NCH + oc + 1) * NCH * 128]
                                lhsT = wsl.rearrange("p (i m) -> p i m", i=NCH)
                                rhs = a4[:, :, rb * RB + kh:rb * RB + kh + RB,
                                         kw:kw + W]
                                nc.tensor.matmul(
                                    out_half, lhsT, rhs,
                                    start=(t == 0), stop=(t == NTAP - 1),
                                    perf_mode=mybir.MatmulPerfMode.DoubleRow)
                    # epilogue on the pair:
                    # r = Relu((1-a)(s p + b0)); v = a s p + (a b0 + b1)
                    # y = r + v + x
                    ep = ept[oc]
                    is_last = (b == BPC - 1 and oc == NCH - 1
                               and pr == NPAIR - 1)
                    # final pair: fine-grained halves, residual on DVE, so
                    # the post-last-matmul chain is short
                    nhalf = 2 if is_last else 1
                    hs = PBE // nhalf
                    yt = work.tile([128, PBE], F32, tag="yt")
                    for h in range(nhalf):
                        sl = slice(h * hs, (h + 1) * hs)
                        r = work.tile([128, hs], F32, tag="r", name="r")
                        nc.scalar.activation(
                            r[:], pt[:, sl], mybir.ActivationFunctionType.Relu,
                            bias=ep[:, 1:2], scale=ep[:, 0:1])
                        v = work.tile([128, hs], F32, tag="v", name="v")
                        nc.vector.tensor_scalar(
                            out=v[:], in0=pt[:, sl], scalar1=ep[:, 2:3],
                            scalar2=ep[:, 3:4], op0=mybir.AluOpType.mult,
                            op1=mybir.AluOpType.add)
                        nc.vector.tensor_add(out=yt[:, sl], in0=r[:], in1=v[:])
                        xsl = xt[b, oc][:, pr * PBE + h * hs:
                                        pr * PBE + (h + 1) * hs]
                        if is_last:
                            nc.vector.tensor_add(out=yt[:, sl], in0=yt[:, sl],
                                                 in1=xsl)
                        else:
                            nc.gpsimd.tensor_add(out=yt[:, sl], in0=yt[:, sl],
                                                 in1=xsl)
                        nc.sync.dma_start(
                            out=y_flat[b, oc * 128:(oc + 1) * 128,
                                       pr * PBE + h * hs:
                                       pr * PBE + (h + 1) * hs],
                            in_=yt[:, sl])


def _pack_inputs(x, move0_bias, conv_w, pr_bias0, prelu_alpha, pr_bias1):
    """Host-side prep: weight binarization + epilogue constant folding."""
    f32 = np.float32
    w = conv_w.astype(f32)
    scale = np.abs(w).mean(axis=(1, 2, 3)).astype(f32)          # (O,)
    ws = np.sign(w).astype(ml_dtypes.float8_e4m3)               # (O,I,KH,KW)
    # lhsT[k=p, tap, oc, ic, m] = ws[oc*128+m, ic*128+p, kh, kw]
    wsr = ws.reshape(NCH, 128, NCH, 128, NTAP)                  # (oc,m,ic,p,t)
    lhsT = wsr.transpose(3, 4, 0, 2, 1)                         # (p,t,oc,ic,m)
    lhsT = np.ascontiguousarray(lhsT).reshape(128, NTAP * NCH * NCH * 128)

    alpha = prelu_alpha.astype(f32).reshape(C)
    b0 = pr_bias0.astype(f32).reshape(C)
    b1 = pr_bias1.astype(f32).reshape(C)
    if USE_LRELU:
        global LRELU_ALPHA_IMM
        assert np.all(alpha == alpha[0]), "Lrelu path needs uniform alpha"
        LRELU_ALPHA_IMM = float(alpha[0])
        epi = np.stack([scale, b0, alpha, b1], axis=1).astype(f32)
    else:
        assert np.all(alpha < 1.0)
        epi = np.stack([(1 - alpha) * scale, (1 - alpha) * b0,
                        alpha * scale, alpha * b0 + b1], axis=1).astype(f32)
    mb = move0_bias.astype(f32).reshape(C, 1)
    skip_b1 = bool(np.all(b1 == 0.0)) if USE_LRELU else False

    common = {"w": lhsT, "mb": mb, "epi": epi}
    in_maps = []
    for i in range(N_CORES):
        m = dict(common)
        m["x"] = np.ascontiguousarray(x[i * BPC:(i + 1) * BPC].astype(f32))
        in_maps.append(m)
    return in_maps, skip_b1


def kernel(x, move0_bias, conv_w, pr_bias0, prelu_alpha, pr_bias1):
    in_maps, skip_b1 = _pack_inputs(
        np.asarray(x), np.asarray(move0_bias), np.asarray(conv_w),
        np.asarray(pr_bias0), np.asarray(prelu_alpha), np.asarray(pr_bias1))
    key = ("nc", skip_b1)
    if key not in _CACHE:
        _CACHE[key] = _build_program(skip_b1)
    nc = _CACHE[key]
    res = bass_utils.run_bass_kernel_spmd(nc, in_maps,
                                          core_ids=list(range(N_CORES)))
    _CACHE["last_results"] = res
    out = np.concatenate([res.results[i]["y"] for i in range(N_CORES)], axis=0)
    return out
